# revision 1
# baseline (speedup 1.0000x reference)
"""Two-layer GAT forward on 8 Trainium2 NeuronCores.

Strategy: edges are partitioned by destination node across the 8 cores
(1250 dst nodes per core), sorted by destination.  Node features are
replicated for the layer-1 GEMM (bf16).  Per-edge feature rows are fetched
with dma_gather; the segment softmax + scatter-add is computed as one-hot
segment matmuls on the TensorEngine accumulating in PSUM per 128-dst
window.  Per-edge destination scores come from a K=1 broadcast matmul +
is_equal one-hot + small segment matmul (dst scores are core-local), so
only one gather stream per layer is needed.  An AllGather shares layer-1
activations (projected through W2) between the layers.  Each core writes
its 1250x128 output slice.

Self-contained: hardcodes the problem shapes from the spec.
"""
import os
import sys
import numpy as np

try:
    import concourse.bass as bass  # noqa
except ImportError:
    sys.path.insert(0, "/opt/trn_rl_repo")

import concourse.bass as bass
import concourse.tile as tile
from concourse import mybir, bacc
from concourse.bass_utils import run_bass_kernel_spmd

# ---------------------------------------------------------------- problem dims
N, E = 10000, 160000
FIN, H1, C1, C2 = 256, 8, 128, 128
D1 = H1 * C1  # 1024
SLOPE = 0.2
NDEV = 8
NLOC = N // NDEV  # 1250
NW = (NLOC + 127) // 128  # 10 windows of 128 dsts (last = 98)
LASTW = NLOC - (NW - 1) * 128  # 98
NT = (N + 127) // 128  # 79 node tiles for the replicated GEMM

F32 = mybir.dt.float32
BF16 = mybir.dt.bfloat16
I16 = mybir.dt.int16
ALU = mybir.AluOpType
ACTF = mybir.ActivationFunctionType

# hcat row layout (bf16 elems): [ h 1024 | s_src 8xf32 (slots 1024:1040) |
#   s_dst 8xf32 (slots 1040:1056) | pad ] -> 1152 bf16 = 2304B
ROW1 = 1152
# layer-2 row (bf16): [ h2 128 | s2src f32 (slots 128:130) | pad ] -> 256 = 512B
ROW2 = 256

_EPS = 1e-30

# --------------------------------------------------------------------- patches


def _apply_drain_patch():
    """This walrus build rejects >1 sync-wait on the Tile-exit Drain; split the
    waits across consecutive drains (semantically identical)."""
    from concourse.vector_clock import ScopedClock

    def _patched(self, tick_clock, wait_clock):
        drain_inst = self.nc.sync.drain()
        wait_clock.add_sem_waits(
            drain_inst.ins, ScopedClock({None: tick_clock.global_clock})
        )
        si = drain_inst.ins.sync_info
        if si is not None and len(si.on_wait) > 1:
            waits = list(si.on_wait)
            si.on_wait = waits[:1]
            drain_inst.ins.sync_info = si
            for i in range(1, len(waits)):
                extra = self.nc.sync.drain()
                esi = extra.ins.sync_info
                if esi is None:
                    esi = mybir.SyncInfo(on_wait=[], on_update=[])
                esi.on_wait = list(esi.on_wait) + waits[i : i + 1]
                extra.ins.sync_info = esi
        self.nc.all_engine_barrier()
        assert self.sems is not None
        popped = self.nc._tile_sem_poison_stack.pop()
        assert popped is self._sem_poison
        self.nc.clear_and_free_semaphores(list(self.sems.allocated().values()))
        self.nc.all_engine_barrier()

    tile.TileContext._drain_and_barrier = _patched


_apply_drain_patch()

# ------------------------------------------------------------------- host prep


def _wrap_idx(idx):
    """dma_gather index layout: idx i at partition i%16, col i//16, replicated
    8x across the 128 partitions."""
    a = np.ascontiguousarray(idx.astype(np.int16).reshape(-1, 16).T)
    return np.ascontiguousarray(np.tile(a, (8, 1)))


def _bf(a):
    import ml_dtypes

    return np.ascontiguousarray(a).astype(ml_dtypes.bfloat16)


def _prep_edges(edge_index):
    """Shard self-loop-augmented edges by dst across devices, sort by dst,
    pad each 128-dst window to a uniform chunk count."""
    loops = np.arange(N, dtype=np.int64)
    src = np.concatenate([np.asarray(edge_index[0], np.int64), loops])
    dst = np.concatenate([np.asarray(edge_index[1], np.int64), loops])

    per_dev = []
    max_chunks = 1
    for d in range(NDEV):
        base = d * NLOC
        sel = (dst >= base) & (dst < base + NLOC)
        s_d, t_d = src[sel], dst[sel]
        order = np.argsort(t_d, kind="stable")
        s_d, t_d = s_d[order], t_d[order]
        wid = (t_d - base) // 128
        cnts = np.bincount(wid, minlength=NW)
        max_chunks = max(max_chunks, int(np.max((cnts + 127) // 128)))
        per_dev.append((s_d, t_d, cnts))

    G = (max_chunks + 1) // 2  # chunks per gather group
    CPW = 2 * G  # uniform chunks per window
    TOT = NW * CPW * 128  # padded edges per device
    NCHUNK = NW * CPW

    devs = []
    for d in range(NDEV):
        base = d * NLOC
        s_d, t_d, cnts = per_dev[d]
        srcp = np.zeros(TOT, np.int64)
        dloc = np.full(TOT, -1.0, np.float32)
        starts = np.concatenate([[0], np.cumsum(cnts)])
        for w in range(NW):
            a, b = starts[w], starts[w + 1]
            o = w * CPW * 128
            n = b - a
            srcp[o : o + n] = s_d[a:b]
            dloc[o : o + n] = (t_d[a:b] - base - w * 128).astype(np.float32)
        # window-dst gather indices (row n of hcat for each window slot)
        widx = np.zeros(NW * 128, np.int64)
        for w in range(NW):
            wl = 128 if w < NW - 1 else LASTW
            widx[w * 128 : w * 128 + wl] = base + w * 128 + np.arange(wl)
        devs.append(
            {
                "srcidx": _wrap_idx(srcp),
                "winidx": _wrap_idx(widx),
                "dstloc": _bf(dloc.reshape(NCHUNK, 128).T),
                "dstrow": _bf(dloc[None, :]),
            }
        )
    return devs, G, CPW


# -------------------------------------------------------------- program build

_CACHE = {}

PHASES = os.environ.get("KPHASES", "ABCD")


def _build(G, CPW, add_b1, add_b2):
    phases = PHASES
    NCHUNK = NW * CPW
    TOT = NCHUNK * 128

    nc = bacc.Bacc()
    dp = nc.declare_dram_parameter
    # shared inputs
    xT_d = dp("xT", [FIN, N], BF16, isOutput=False)
    W1_d = dp("W1aug", [FIN, D1 + 16], BF16, isOutput=False)
    W2_d = dp("W2aug", [D1, C2 + 2], BF16, isOutput=False)
    iota_d = dp("iota_rep", [128, G * 128], BF16, isOutput=False)
    iotac_d = dp("iota_col", [128, 1], F32, isOutput=False)
    ones_d = dp("ones_row", [1, 128], BF16, isOutput=False)
    ident_d = dp("ident", [128, 128], BF16, isOutput=False)
    b1_d = dp("b1bc", [128, D1], F32, isOutput=False)
    b2_d = dp("b2bc", [128, C2], F32, isOutput=False)
    # per-device inputs
    srcidx_d = dp("srcidx", [128, TOT // 16], I16, isOutput=False)
    winidx_d = dp("winidx", [128, NW * 128 // 16], I16, isOutput=False)
    dstloc_d = dp("dstloc", [128, NCHUNK], BF16, isOutput=False)
    dstrow_d = dp("dstrow", [1, TOT], BF16, isOutput=False)
    # output
    out_d = dp("out", [NLOC, C2], F32, isOutput=True)
    dbg = {}
    if "B" not in phases:
        dbg["hcat"] = dp("dbg_hcat", [N, ROW1], BF16, isOutput=True)
    elif "C" not in phases:
        dbg["h1"] = dp("dbg_h1", [NLOC, D1], F32, isOutput=True)
    elif "D" not in phases:
        dbg["h2all"] = dp("dbg_h2all", [N, ROW2], BF16, isOutput=True)
    # internal DRAM
    hcat = nc.dram_tensor("hcat", [N, ROW1], BF16)
    h2loc = nc.dram_tensor("h2loc", [NLOC, ROW2], BF16)
    h2all = nc.dram_tensor("h2all", [N, ROW2], BF16, addr_space="Shared")

    with tile.TileContext(nc) as tc:
        with tc.tile_pool(name="const", bufs=1) as constp:
            iota_t = constp.tile([128, G * 128], BF16)
            nc.sync.dma_start(iota_t[:], iota_d[:])
            iotac_t = constp.tile([128, 1], F32)
            nc.sync.dma_start(iotac_t[:], iotac_d[:])
            ones_t = constp.tile([1, 128], BF16)
            nc.sync.dma_start(ones_t[:], ones_d[:])
            ident_t = constp.tile([128, 128], BF16)
            nc.sync.dma_start(ident_t[:], ident_d[:])
            b1_t = constp.tile([128, D1], F32)
            if add_b1:
                nc.sync.dma_start(b1_t[:], b1_d[:])
            b2_t = constp.tile([128, C2], F32)
            if add_b2:
                nc.sync.dma_start(b2_t[:], b2_d[:])
            srcidx_t = constp.tile([128, TOT // 16], I16)
            nc.sync.dma_start(srcidx_t[:], srcidx_d[:])
            winidx_t = constp.tile([128, NW * 128 // 16], I16)
            nc.sync.dma_start(winidx_t[:], winidx_d[:])
            dstloc_t = constp.tile([128, NCHUNK], BF16)
            nc.sync.dma_start(dstloc_t[:], dstloc_d[:])
            dstrow_t = constp.tile([1, TOT], BF16)
            nc.sync.dma_start(dstrow_t[:], dstrow_d[:])
            h1T_t = constp.tile([128, 8, NW * 128], BF16)
            s2dcol_t = constp.tile([128, NW], F32)
            nc.vector.memset(s2dcol_t[:], 0.0)

            # ---------------- Phase A: replicated h = x @ W1aug + score cols
            with (
                tc.tile_pool(name="gemmA", bufs=1) as gA,
                tc.tile_pool(name="outA", bufs=3) as oA,
                tc.tile_pool(name="psA", bufs=3, space="PSUM") as psA_p,
                tc.tile_pool(name="psAs", bufs=2, space="PSUM") as psAs_p,
            ):
                xT_t = gA.tile([128, 2, N], BF16)
                nc.sync.dma_start(
                    xT_t[:], xT_d[:].rearrange("(k p) n -> p k n", p=128)
                )
                W1_t = gA.tile([128, 2, D1 + 16], BF16)
                nc.sync.dma_start(
                    W1_t[:], W1_d[:].rearrange("(k p) f -> p k f", p=128)
                )
                for t in range(NT):
                    tl = min(128, N - t * 128)
                    ps = psA_p.tile([128, 1024], F32, tag="psA")
                    pss = psAs_p.tile([128, 16], F32, tag="psAs")
                    for k in range(2):
                        lhsT = xT_t[:, k, t * 128 : t * 128 + tl]
                        nc.tensor.matmul(
                            ps[:tl, 0:512], lhsT, W1_t[:, k, 0:512],
                            start=(k == 0), stop=(k == 1),
                        )
                        nc.tensor.matmul(
                            ps[:tl, 512:1024], lhsT, W1_t[:, k, 512:1024],
                            start=(k == 0), stop=(k == 1),
                        )
                        nc.tensor.matmul(
                            pss[:tl, 0:16], lhsT, W1_t[:, k, 1024:1040],
                            start=(k == 0), stop=(k == 1),
                        )
                    hc = oA.tile([128, ROW1], BF16, tag="hc")
                    nc.vector.tensor_copy(hc[:tl, 0:512], ps[:tl, 0:512])
                    nc.scalar.activation(
                        hc[:tl, 512:1024], ps[:tl, 512:1024], ACTF.Copy
                    )
                    nc.vector.tensor_copy(
                        hc[:tl].bitcast(F32)[:, 512:528], pss[:tl, 0:16]
                    )
                    nc.sync.dma_start(
                        hcat[t * 128 : t * 128 + tl, :], hc[:tl, :]
                    )

            if "B" not in phases:
                nc.sync.dma_start(dbg["hcat"][:, :], hcat[:, :])

            # ---------------- Phase B: layer-1 edge aggregation
            if "B" in phases:
                with (
                    tc.tile_pool(name="edgeB", bufs=2) as eB,
                    tc.tile_pool(name="winB", bufs=2) as wB,
                    tc.tile_pool(name="psw", bufs=2, space="PSUM") as psw_p,
                    tc.tile_pool(name="den", bufs=1, space="PSUM") as den_p,
                    tc.tile_pool(name="aux", bufs=1, space="PSUM") as aux_p,
                    tc.tile_pool(name="esp", bufs=1, space="PSUM") as es_p,
                ):
                    for w in range(NW):
                        wl = 128 if w < NW - 1 else LASTW
                        psw = psw_p.tile([128, 1024], F32, tag="psw")
                        den = den_p.tile([128, 8], F32, tag="den")
                        # window dst rows -> s_dst values for this window
                        swin = eB.tile([128, 1, ROW1], BF16, tag="swin")
                        nc.gpsimd.dma_gather(
                            out_ap=swin[:], in_ap=hcat[:, :],
                            idxs_ap=winidx_t[:, w * 8 : w * 8 + 8],
                            num_idxs=128, num_idxs_reg=128,
                            elem_size=ROW1, single_packet=False,
                        )
                        sdw = swin[:, 0].bitcast(F32)[:, 520:528]  # [128,8]
                        for g in range(2):
                            k0 = w * CPW + g * G
                            o16 = k0 * 8
                            hg = eB.tile([128, G, ROW1], BF16, tag="hg")
                            gh = (G + 1) // 2
                            for hv in range(2):
                                c0, c1 = hv * gh, min(G, (hv + 1) * gh)
                                if c0 >= c1:
                                    continue
                                nc.gpsimd.dma_gather(
                                    out_ap=hg[:, c0:c1, :], in_ap=hcat[:, :],
                                    idxs_ap=srcidx_t[:, o16 + c0 * 8 : o16 + c1 * 8],
                                    num_idxs=(c1 - c0) * 128,
                                    num_idxs_reg=(c1 - c0) * 128,
                                    elem_size=ROW1, single_packet=True,
                                )
                            # broadcast dst slots + build S (d-major one-hot),
                            # then per-edge sdst via segment matmul
                            esp = es_p.tile([128, G * 8], F32, tag="esp")
                            Ssb = wB.tile([128, G * 128], F32, tag="Ssb")
                            for cb in range(0, G, 4):
                                nb = min(4, G - cb)
                                bc = aux_p.tile([128, 512], F32, tag="aux")
                                nc.tensor.matmul(
                                    bc[:, 0 : nb * 128], ones_t[:],
                                    dstrow_t[0:1, (k0 + cb) * 128 : (k0 + cb + nb) * 128],
                                    start=True, stop=True,
                                )
                                nc.vector.tensor_scalar(
                                    Ssb[:, cb * 128 : (cb + nb) * 128],
                                    bc[:, 0 : nb * 128], iotac_t[:, 0:1],
                                    None, ALU.is_equal,
                                )
                            for c in range(G):
                                nc.tensor.matmul(
                                    esp[:, c * 8 : c * 8 + 8],
                                    Ssb[:, c * 128 : (c + 1) * 128],
                                    sdw, start=True, stop=True,
                                )
                            # p = exp(leaky_relu(ssrc + sdst))  [128, G, 8]
                            pt = eB.tile([128, G, 8], F32, tag="pt")
                            nc.vector.tensor_tensor(
                                pt[:],
                                hg[:].bitcast(F32)[:, :, 512:520],
                                esp[:].rearrange("e (g h) -> e g h", h=8),
                                ALU.add,
                            )
                            lt = eB.tile([128, G, 8], F32, tag="lt")
                            nc.vector.tensor_scalar(
                                lt[:], pt[:], SLOPE, None, ALU.mult
                            )
                            nc.vector.tensor_tensor(pt[:], pt[:], lt[:], ALU.max)
                            pb = eB.tile([128, G, 8], BF16, tag="pb")
                            nc.scalar.activation(pb[:], pt[:], ACTF.Exp)
                            # h *= p (per-head broadcast)
                            h4 = hg[:, :, 0:1024].rearrange(
                                "e g (h c) -> e g h c", c=C1
                            )
                            p4 = pb[:].unsqueeze(3).broadcast_to([128, G, 8, C1])
                            nc.vector.tensor_tensor(h4, h4, p4, ALU.mult)
                            # S^T (e-major one-hot), batched over the group
                            stg = wB.tile([128, G * 128], BF16, tag="stg")
                            nc.vector.tensor_tensor(
                                stg[:].rearrange("e (g d) -> e g d", d=128),
                                iota_t[:].rearrange("e (g d) -> e g d", d=128),
                                dstloc_t[:, k0 : k0 + G]
                                .unsqueeze(2)
                                .broadcast_to([128, G, 128]),
                                ALU.is_equal,
                            )
                            first = g == 0
                            last = g == 1
                            for c in range(G):
                                st = stg[:, c * 128 : (c + 1) * 128]
                                fc = first and c == 0
                                lc = last and c == G - 1
                                nc.tensor.matmul(
                                    den[:], st, pb[:, c, :],
                                    start=fc, stop=lc,
                                )
                                nc.tensor.matmul(
                                    psw[:, 0:512], st, hg[:, c, 0:512],
                                    start=fc, stop=lc,
                                )
                                nc.tensor.matmul(
                                    psw[:, 512:1024], st, hg[:, c, 512:1024],
                                    start=fc, stop=lc,
                                )
                        # window epilogue: h1 = elu(agg/denom + b1); h1T via PE
                        dens = wB.tile([128, 8], F32, tag="dens")
                        nc.vector.tensor_scalar(
                            dens[:], den[:], _EPS, None, ALU.max
                        )
                        rec = wB.tile([128, 8], F32, tag="rec")
                        nc.vector.reciprocal(rec[:], dens[:])
                        h1r = wB.tile([128, D1], F32, tag="h1r")
                        for half in range(2):
                            o = 512 * half
                            nc.vector.tensor_tensor(
                                h1r[:, o : o + 512].rearrange(
                                    "e (h c) -> e h c", c=C1
                                ),
                                psw[:, o : o + 512].rearrange(
                                    "e (h c) -> e h c", c=C1
                                ),
                                rec[:, 4 * half : 4 * half + 4]
                                .unsqueeze(2)
                                .broadcast_to([128, 4, C1]),
                                ALU.mult,
                            )
                        if add_b1:
                            nc.vector.tensor_tensor(
                                h1r[:], h1r[:], b1_t[:], ALU.add
                            )
                        etmp = wB.tile([128, D1], F32, tag="etmp")
                        nc.scalar.activation(etmp[:], h1r[:], ACTF.Exp)
                        nc.vector.tensor_scalar(
                            etmp[:], etmp[:], 1.0, 0.0, ALU.subtract, ALU.min
                        )
                        nc.vector.tensor_scalar(
                            h1r[:], h1r[:], 0.0, None, ALU.max
                        )
                        h1b = wB.tile([128, D1], BF16, tag="h1b")
                        nc.vector.tensor_tensor(h1b[:], h1r[:], etmp[:], ALU.add)
                        for j in range(8):
                            tp = den_p.tile([128, 128], BF16, tag="tp")
                            nc.tensor.transpose(
                                tp[:, 0:128], h1b[:, j * 128 : (j + 1) * 128],
                                ident_t[:],
                            )
                            nc.vector.tensor_copy(
                                h1T_t[:, j, w * 128 : w * 128 + wl],
                                tp[:, 0:wl],
                            )
                        if "C" not in phases:
                            nc.vector.tensor_copy(
                                etmp[:], h1b[:]
                            )
                            nc.sync.dma_start(
                                dbg["h1"][w * 128 : w * 128 + wl, :],
                                etmp[:wl, :],
                            )

            # ---------------- Phase C: h2 = h1 @ W2aug (sharded) + AllGather
            if "C" in phases:
                with (
                    tc.tile_pool(name="gemmC", bufs=3) as gC,
                    tc.tile_pool(name="psC", bufs=4, space="PSUM") as psC_p,
                ):
                    W2_t = gC.tile([128, 8, C2 + 2], BF16, tag="w2")
                    nc.sync.dma_start(
                        W2_t[:], W2_d[:].rearrange("(k p) f -> p k f", p=128)
                    )
                    for w in range(NW):
                        wl = 128 if w < NW - 1 else LASTW
                        ps2 = psC_p.tile([128, 512], F32, tag="ps2")
                        for k in range(8):
                            nc.tensor.matmul(
                                ps2[:wl, 0 : C2 + 2],
                                h1T_t[:, k, w * 128 : w * 128 + wl],
                                W2_t[:, k, :],
                                start=(k == 0), stop=(k == 7),
                            )
                        h2t = gC.tile([128, ROW2], BF16, tag="h2t")
                        nc.vector.tensor_copy(h2t[:wl, 0:128], ps2[:wl, 0:128])
                        nc.vector.tensor_copy(
                            h2t[:wl].bitcast(F32)[:, 64:65], ps2[:wl, 128:129]
                        )
                        nc.vector.tensor_copy(
                            s2dcol_t[:wl, w : w + 1], ps2[:wl, 129:130]
                        )
                        nc.sync.dma_start(
                            h2loc[w * 128 : w * 128 + wl, :], h2t[:wl, :]
                        )
                nc.gpsimd.collective_compute(
                    "AllGather",
                    ALU.bypass,
                    ins=[h2loc[:]],
                    outs=[h2all[:]],
                    replica_groups=[list(range(NDEV))],
                )
                if "D" not in phases:
                    nc.sync.dma_start(dbg["h2all"][:, :], h2all[:, :])

            # ---------------- Phase D: layer-2 edge aggregation
            if "D" in phases:
                with (
                    tc.tile_pool(name="edgeD", bufs=2) as eD,
                    tc.tile_pool(name="winD", bufs=2) as wD,
                    tc.tile_pool(name="psw2", bufs=2, space="PSUM") as psw2_p,
                    tc.tile_pool(name="den2", bufs=1, space="PSUM") as den2_p,
                    tc.tile_pool(name="aux2", bufs=1, space="PSUM") as aux2_p,
                    tc.tile_pool(name="esp2", bufs=1, space="PSUM") as es2_p,
                ):
                    for w in range(NW):
                        wl = 128 if w < NW - 1 else LASTW
                        psw2 = psw2_p.tile([128, 512], F32, tag="psw2")
                        den2 = den2_p.tile([128, 8], F32, tag="den2")
                        for g in range(2):
                            k0 = w * CPW + g * G
                            o16 = k0 * 8
                            g2 = eD.tile([128, G, ROW2], BF16, tag="g2")
                            gh = (G + 1) // 2
                            for hv in range(2):
                                c0, c1 = hv * gh, min(G, (hv + 1) * gh)
                                if c0 >= c1:
                                    continue
                                nc.gpsimd.dma_gather(
                                    out_ap=g2[:, c0:c1, :], in_ap=h2all[:, :],
                                    idxs_ap=srcidx_t[:, o16 + c0 * 8 : o16 + c1 * 8],
                                    num_idxs=(c1 - c0) * 128,
                                    num_idxs_reg=(c1 - c0) * 128,
                                    elem_size=ROW2, single_packet=True,
                                )
                            esp = es2_p.tile([128, G * 8], F32, tag="esp2")
                            Ssb = wD.tile([128, G * 128], F32, tag="Ssb2")
                            for cb in range(0, G, 4):
                                nb = min(4, G - cb)
                                bc = aux2_p.tile([128, 512], F32, tag="aux2")
                                nc.tensor.matmul(
                                    bc[:, 0 : nb * 128], ones_t[:],
                                    dstrow_t[0:1, (k0 + cb) * 128 : (k0 + cb + nb) * 128],
                                    start=True, stop=True,
                                )
                                nc.vector.tensor_scalar(
                                    Ssb[:, cb * 128 : (cb + nb) * 128],
                                    bc[:, 0 : nb * 128], iotac_t[:, 0:1],
                                    None, ALU.is_equal,
                                )
                            for c in range(G):
                                nc.tensor.matmul(
                                    esp[:, c * 8 : c * 8 + 1],
                                    Ssb[:, c * 128 : (c + 1) * 128],
                                    s2dcol_t[:, w : w + 1],
                                    start=True, stop=True,
                                )
                            pt = eD.tile([128, G, 1], F32, tag="pt2")
                            nc.vector.tensor_tensor(
                                pt[:],
                                g2[:].bitcast(F32)[:, :, 64:65],
                                esp[:].rearrange("e (g h) -> e g h", h=8)[:, :, 0:1],
                                ALU.add,
                            )
                            lt = eD.tile([128, G, 1], F32, tag="lt2")
                            nc.vector.tensor_scalar(
                                lt[:], pt[:], SLOPE, None, ALU.mult
                            )
                            nc.vector.tensor_tensor(pt[:], pt[:], lt[:], ALU.max)
                            pb = eD.tile([128, G, 1], BF16, tag="pb2")
                            nc.scalar.activation(pb[:], pt[:], ACTF.Exp)
                            nc.vector.tensor_tensor(
                                g2[:, :, 0:128], g2[:, :, 0:128],
                                pb[:].broadcast_to([128, G, 128]),
                                ALU.mult,
                            )
                            stg = wD.tile([128, G * 128], BF16, tag="stg2")
                            nc.vector.tensor_tensor(
                                stg[:].rearrange("e (g d) -> e g d", d=128),
                                iota_t[:].rearrange("e (g d) -> e g d", d=128),
                                dstloc_t[:, k0 : k0 + G]
                                .unsqueeze(2)
                                .broadcast_to([128, G, 128]),
                                ALU.is_equal,
                            )
                            for c in range(G):
                                st = stg[:, c * 128 : (c + 1) * 128]
                                fc = g == 0 and c == 0
                                lc = g == 1 and c == G - 1
                                nc.tensor.matmul(
                                    den2[:, 0:1], st, pb[:, c, :],
                                    start=fc, stop=lc,
                                )
                                nc.tensor.matmul(
                                    psw2[:, 0:128], st, g2[:, c, 0:128],
                                    start=fc, stop=lc,
                                )
                        dens = wD.tile([128, 1], F32, tag="dens2")
                        nc.vector.tensor_scalar(
                            dens[:], den2[:, 0:1], _EPS, None, ALU.max
                        )
                        rec2 = wD.tile([128, 1], F32, tag="rec2")
                        nc.vector.reciprocal(rec2[:], dens[:])
                        ot = wD.tile([128, C2], F32, tag="ot")
                        nc.vector.tensor_scalar(
                            ot[:], psw2[:, 0:128], rec2[:, 0:1], None, ALU.mult
                        )
                        if add_b2:
                            nc.vector.tensor_tensor(ot[:], ot[:], b2_t[:], ALU.add)
                        nc.sync.dma_start(
                            out_d[w * 128 : w * 128 + wl, :], ot[:wl, :]
                        )

    nc.finalize()
    return nc


# ------------------------------------------------------------------ entrypoint

TRACE = [False]
LAST = [None]


def kernel(x, edge_index, W1, a_src1, a_dst1, b1, W2, a_src2, a_dst2, b2):
    x = np.asarray(x, np.float32)
    W1 = np.asarray(W1, np.float32)
    W2 = np.asarray(W2, np.float32)
    a_src1 = np.asarray(a_src1, np.float32)
    a_dst1 = np.asarray(a_dst1, np.float32)
    a_src2 = np.asarray(a_src2, np.float32)
    a_dst2 = np.asarray(a_dst2, np.float32)
    b1 = np.asarray(b1, np.float32)
    b2 = np.asarray(b2, np.float32)
    ei = np.asarray(edge_index)

    devs, G, CPW = _prep_edges(ei)

    # fold attention projections into the GEMM weights
    A1 = np.zeros((D1, 16), np.float32)
    for h in range(H1):
        A1[h * C1 : (h + 1) * C1, h] = a_src1[h]
        A1[h * C1 : (h + 1) * C1, 8 + h] = a_dst1[h]
    import ml_dtypes

    bf = lambda a: np.ascontiguousarray(a).astype(ml_dtypes.bfloat16)
    W1aug = np.concatenate([W1, W1 @ A1], axis=1)
    W2aug = np.concatenate(
        [W2, W2 @ a_src2[0][:, None], W2 @ a_dst2[0][:, None]], 1
    )
    add_b1 = bool(np.any(b1 != 0))
    add_b2 = bool(np.any(b2 != 0))

    key = (G, CPW, add_b1, add_b2, PHASES)
    if key not in _CACHE:
        _CACHE[key] = _build(G, CPW, add_b1, add_b2)
    nc = _CACHE[key]

    shared = {
        "xT": bf(x.T),
        "W1aug": bf(W1aug),
        "W2aug": bf(W2aug),
        "iota_rep": bf(
            np.tile(np.arange(128, dtype=np.float32)[None, :], (128, G))
        ),
        "iota_col": np.arange(128, dtype=np.float32)[:, None].copy(),
        "ones_row": bf(np.ones((1, 128), np.float32)),
        "ident": bf(np.eye(128, dtype=np.float32)),
        "b1bc": np.ascontiguousarray(np.tile(b1[None, :], (128, 1))),
        "b2bc": np.ascontiguousarray(np.tile(b2[None, :], (128, 1))),
    }
    in_maps = [{**shared, **devs[d]} for d in range(NDEV)]
    res = run_bass_kernel_spmd(nc, in_maps, list(range(NDEV)), trace=TRACE[0])
    LAST[0] = res
    out = np.concatenate([res.results[d]["out"] for d in range(NDEV)], axis=0)
    return out.astype(np.float32)

